# revision 21
# baseline (speedup 1.0000x reference)
"""Llama decoder layer (S=4096, D=768, NH=12, I=3072, fp32) on 8 TRN2 cores.

Strategy: sequence-sharded data parallel. Each core owns 512 rows:
  rmsnorm1 -> QKV (bf16 matmuls, ln folded into weights host-side; all
  weights uploaded as bf16 to halve HBM traffic)
  -> AllGather K^T and V (bf16) -> full-softmax attention in transposed
  layout (scoresT [k,q]; exp with no max-subtract — scores are O(1);
  per-head double-buffered 2-bank scores PSUM so exp overlaps the next
  scores matmul; AV accumulates in PSUM across all 16 k-groups in one
  start/stop chain; softmax denominator via a ones-column appended to V;
  Z broadcast via a tiny PE matmul) -> o_proj + residual -> rmsnorm2 ->
  MLP (silu) -> residual. The K/V AllGather is split in two: the K-gather
  is issued right after the K projection and overlaps the Q/V projection
  matmuls (scores need only K; AV needs V later). Host re-assembles the
  8 output shards. Marginal device time ~0.7ms/layer by reps=4 slope;
  split-AG measured -42us/layer vs single-AG in same-process A/B.
"""
import numpy as np
import ml_dtypes  # noqa: F401  (bf16 host checks)
import concourse.bass as bass
import concourse.tile as tile
from concourse import bacc, mybir
from concourse import masks
from concourse.bass_utils import run_bass_kernel_spmd

dt = mybir.dt
AF = mybir.ActivationFunctionType

N_CORES = 8
S, D, NH, HD, I = 4096, 768, 12, 64, 3072
SH = S // N_CORES          # 512 rows per core
NSB = SH // 128            # 4 s-blocks per core
NKT = D // 128             # 6 contraction tiles over D
NIT = I // 128             # 24 i-tiles
PAIRS = NH // 2            # 6 head pairs
VW = 65 * NH               # 780: V row width with ones col per head
KT_ELEMS = D * SH          # KT shard elems (bf16)
V_ELEMS = SH * VW
KV_ELEMS = KT_ELEMS + V_ELEMS
EPS = 1e-5
SCALE = 1.0 / np.sqrt(HD)

_CACHED = {}


def _build(reps=1):
    # reps>1 unrolls the whole layer body N times in one NEFF — used only
    # for slope-calibration timing (time(2x)-time(1x) cancels launch
    # overhead); kernel() always runs the reps=1 build.
    nc = bacc.Bacc("TRN2", target_bir_lowering=False, debug=False,
                   enable_asserts=False, num_devices=N_CORES)
    Xs = nc.dram_tensor("x_shard", [SH, D], dt.float32, kind="ExternalInput")
    WQ = nc.dram_tensor("wqT", [D, D], dt.bfloat16, kind="ExternalInput")
    WK = nc.dram_tensor("wkT", [D, D], dt.bfloat16, kind="ExternalInput")
    WV = nc.dram_tensor("wvT", [D, D], dt.bfloat16, kind="ExternalInput")
    WO = nc.dram_tensor("woT", [D, D], dt.bfloat16, kind="ExternalInput")
    WU = nc.dram_tensor("wupT", [D, I], dt.bfloat16, kind="ExternalInput")
    WD = nc.dram_tensor("wdownT", [I, D], dt.bfloat16, kind="ExternalInput")
    OUT = nc.dram_tensor("out", [SH, D], dt.float32, kind="ExternalOutput")

    def rmsnorm_to_xnT(nc, wk_pool, pst, ident, xsrc, xnT_dst, sb):
        sq = wk_pool.tile([128, D], dt.float32, tag="sq", name="sq")
        nc.vector.tensor_mul(sq[:], xsrc, xsrc)
        ssum = wk_pool.tile([128, 1], dt.float32, tag="ssum", name="ssum")
        nc.vector.reduce_sum(ssum[:], sq[:], axis=mybir.AxisListType.X)
        mvar = wk_pool.tile([128, 1], dt.float32, tag="mvar", name="mvar")
        nc.vector.tensor_scalar(out=mvar[:], in0=ssum[:], scalar1=1.0 / D,
                                scalar2=EPS, op0=mybir.AluOpType.mult,
                                op1=mybir.AluOpType.add)
        rvar = wk_pool.tile([128, 1], dt.float32, tag="rvar", name="rvar")
        nc.vector.reciprocal(rvar[:], mvar[:])
        rr = wk_pool.tile([128, 1], dt.float32, tag="rr", name="rr")
        nc.scalar.activation(rr[:], rvar[:], AF.Sqrt)
        xn = wk_pool.tile([128, D], dt.float32, tag="xn", name="xn")
        nc.vector.tensor_scalar_mul(xn[:], xsrc, rr[:])
        for kt in range(NKT):
            t_ps = pst.tile([128, 128], dt.float32, tag="tps", name="t_ps")
            nc.tensor.transpose(t_ps[:], xn[:, kt * 128:(kt + 1) * 128], ident[:])
            nc.vector.tensor_copy(xnT_dst[:, kt * SH + sb * 128: kt * SH + (sb + 1) * 128], t_ps[:])

    with tile.TileContext(nc) as tc:
      for rep in range(reps):
        with tc.tile_pool(name=f"const{rep}", bufs=1) as constp, \
             tc.tile_pool(name=f"mid{rep}", bufs=1) as mid, \
             tc.tile_pool(name=f"dram{rep}", bufs=1, space="DRAM") as dram:

            ident = constp.tile([128, 128], dt.float32)
            masks.make_identity(nc, ident[:])
            ones_f = constp.tile([1, 64], dt.float32)
            nc.gpsimd.memset(ones_f[:], 1.0)
            ones_r = constp.tile([1, 64], dt.float32r)
            nc.vector.tensor_copy(ones_r[:], ones_f[:])

            x_sb = mid.tile([128, NSB * D], dt.float32)
            x2_sb = mid.tile([128, NSB * D], dt.float32)
            qT = mid.tile([128, NKT * SH], dt.bfloat16)
            xnT2 = mid.tile([128, NKT * SH], dt.bfloat16)
            # MLP-up weights live in the outer pool and load during phase A:
            # the first up-proj matmul needs a slice of every kt-chunk, so a
            # phase-C load would expose the whole 4.7MB DMA.
            wup_all = mid.tile([128, NKT * I], dt.bfloat16)
            k_in = dram.tile([KT_ELEMS], dt.bfloat16)
            v_in = dram.tile([V_ELEMS], dt.bfloat16)
            k_all = dram.tile([N_CORES, KT_ELEMS], dt.bfloat16, addr_space="Shared")
            v_all = dram.tile([N_CORES, V_ELEMS], dt.bfloat16, addr_space="Shared")

            # ============ phase A: norm1, QKV, AG ============
            with tc.tile_pool(name="wqkv", bufs=1) as wqkv, \
                 tc.tile_pool(name="workA", bufs=2) as workA, \
                 tc.tile_pool(name="psA", bufs=2, space="PSUM") as psA, \
                 tc.tile_pool(name="pstA", bufs=2, space="PSUM") as pstA:
                wq_all = wqkv.tile([128, NKT * D], dt.bfloat16)
                wk_all = wqkv.tile([128, NKT * D], dt.bfloat16)
                wv_all = wqkv.tile([128, NKT * D], dt.bfloat16)
                for kt in range(NKT):
                    nc.gpsimd.dma_start(wq_all[:, kt * D:(kt + 1) * D], WQ.ap()[kt * 128:(kt + 1) * 128, :])
                    nc.gpsimd.dma_start(wk_all[:, kt * D:(kt + 1) * D], WK.ap()[kt * 128:(kt + 1) * 128, :])
                    nc.gpsimd.dma_start(wv_all[:, kt * D:(kt + 1) * D], WV.ap()[kt * 128:(kt + 1) * 128, :])
                for kt in range(NKT):
                    for ch in range(4):
                        nc.gpsimd.dma_start(
                            wup_all[:, kt * I + ch * 768: kt * I + (ch + 1) * 768],
                            WU.ap()[kt * 128:(kt + 1) * 128, ch * 768:(ch + 1) * 768])

                # V is the only per-s-block projection (x-stationary), so it
                # interleaves into the rmsnorm loop: the PE computes V(sb)
                # while the vector engine runs rmsnorm(sb+1), instead of
                # idling until all four blocks are normed.
                xnT = wqkv.tile([128, NKT * SH], dt.bfloat16)
                v_sh = wqkv.tile([128, NSB * VW], dt.bfloat16)
                nc.gpsimd.memset(v_sh[:], 1.0)
                for sb in range(NSB):
                    xs = x_sb[:, sb * D:(sb + 1) * D]
                    nc.sync.dma_start(xs, Xs.ap()[sb * 128:(sb + 1) * 128, :])
                    rmsnorm_to_xnT(nc, workA, pstA, ident, xs, xnT, sb)
                    for jc in range(2):
                        pp = psA.tile([128, 384], dt.float32, tag="projv", name="pp")
                        for kt in range(NKT):
                            nc.tensor.matmul(pp[:], xnT[:, kt * SH + sb * 128: kt * SH + (sb + 1) * 128],
                                             wv_all[:, kt * D + jc * 384: kt * D + (jc + 1) * 384],
                                             start=(kt == 0), stop=(kt == NKT - 1))
                        for h6 in range(6):
                            h = jc * 6 + h6
                            nc.vector.tensor_copy(
                                v_sh[:, sb * VW + 65 * h: sb * VW + 65 * h + 64],
                                pp[:, h6 * 64:(h6 + 1) * 64])

                # K projection first; its AllGather flies while Q and V
                # projections run on the PE (scores only need K, AV needs V
                # later — the big collective is no longer one barrier).
                kT = wqkv.tile([128, NKT * SH], dt.bfloat16)
                for ob in range(NKT):
                    pp = psA.tile([128, SH], dt.float32, tag="proj", name="pp")
                    for kt in range(NKT):
                        nc.tensor.matmul(pp[:], wk_all[:, kt * D + ob * 128: kt * D + (ob + 1) * 128],
                                         xnT[:, kt * SH:(kt + 1) * SH],
                                         start=(kt == 0), stop=(kt == NKT - 1))
                    nc.vector.tensor_copy(kT[:, ob * SH:(ob + 1) * SH], pp[:])
                for ob in range(NKT):
                    nc.sync.dma_start(
                        k_in[ob * 128 * SH:(ob + 1) * 128 * SH].rearrange("(p c) -> p c", p=128),
                        kT[:, ob * SH:(ob + 1) * SH])
                nc.gpsimd.collective_compute(
                    "AllGather", mybir.AluOpType.bypass,
                    replica_groups=[list(range(N_CORES))],
                    ins=[k_in.opt()], outs=[k_all.opt()])
                for ob in range(NKT):
                    pp = psA.tile([128, SH], dt.float32, tag="proj", name="pp")
                    for kt in range(NKT):
                        nc.tensor.matmul(pp[:], wq_all[:, kt * D + ob * 128: kt * D + (ob + 1) * 128],
                                         xnT[:, kt * SH:(kt + 1) * SH],
                                         start=(kt == 0), stop=(kt == NKT - 1))
                    nc.vector.tensor_copy(qT[:, ob * SH:(ob + 1) * SH], pp[:])
                for sb in range(NSB):
                    nc.sync.dma_start(
                        v_in[sb * 128 * VW: (sb + 1) * 128 * VW]
                        .rearrange("(p c) -> p c", p=128),
                        v_sh[:, sb * VW:(sb + 1) * VW])
            nc.gpsimd.collective_compute(
                "AllGather", mybir.AluOpType.bypass,
                replica_groups=[list(range(N_CORES))],
                ins=[v_in.opt()], outs=[v_all.opt()])

            # ============ phase B: attention + o_proj + norm2 ============
            with tc.tile_pool(name="attB", bufs=1) as attB:
              with tc.tile_pool(name="att_kt", bufs=2) as att_kt, \
                 tc.tile_pool(name="att_e", bufs=3) as att_e, \
                 tc.tile_pool(name="att_sm", bufs=2) as att_sm, \
                 tc.tile_pool(name="att_ps", bufs=2, space="PSUM") as att_ps, \
                 tc.tile_pool(name="o_ps_pool", bufs=1, space="PSUM") as o_ps_pool, \
                 tc.tile_pool(name="z_ps_pool", bufs=1, space="PSUM") as z_ps_pool:
                wo_all = attB.tile([128, NKT * D], dt.bfloat16)
                for kt in range(NKT):
                    nc.gpsimd.dma_start(wo_all[:, kt * D:(kt + 1) * D], WO.ap()[kt * 128:(kt + 1) * 128, :])
                v_full = attB.tile([128, 32 * VW], dt.bfloat16)
                for r in range(N_CORES):
                    for sb in range(NSB):
                        c = r * NSB + sb
                        nc.sync.dma_start(
                            v_full[:, c * VW:(c + 1) * VW],
                            v_all[r, sb * 128 * VW: (sb + 1) * 128 * VW]
                            .rearrange("(p c) -> p c", p=128))
                oT = attB.tile([128, PAIRS * SH], dt.bfloat16)
                for t in range(PAIRS):
                    kt_pair = att_kt.tile([128, S], dt.bfloat16, tag="ktp", name="kt_pair")
                    for r in range(N_CORES):
                        nc.sync.dma_start(
                            kt_pair[:, r * SH:(r + 1) * SH],
                            k_all[r, t * 128 * SH:(t + 1) * 128 * SH]
                            .rearrange("(p c) -> p c", p=128))
                    # AV accumulates directly in PSUM across all 16 k-groups
                    # (one start/stop chain per head) — no per-group SBUF
                    # round-trip / vector adds on the critical path.
                    o_acc0 = o_ps_pool.tile([65, SH], dt.float32, tag="oacc0", name="o_acc0", bufs=1)
                    o_acc1 = o_ps_pool.tile([65, SH], dt.float32, tag="oacc1", name="o_acc1", bufs=1)
                    o_accs = (o_acc0, o_acc1)
                    for g in range(16):
                        # per-head 2-bank scores psum, double-buffered: exp(g,hh)
                        # overlaps the PE's next scores matmul instead of
                        # serializing PE -> scalar -> PE each group.
                        for hh in range(2):
                            b = 64 * hh
                            s_ps = att_ps.tile([128, 1024], dt.float32, tag="sps", name="s_ps", bufs=2)
                            for j in range(2):
                                kb = g * 2 + j
                                nc.tensor.matmul(
                                    s_ps[:, j * SH:(j + 1) * SH],
                                    kt_pair[b:b + 64, kb * 128:(kb + 1) * 128],
                                    qT[b:b + 64, t * SH:(t + 1) * SH],
                                    start=True, stop=True)
                            eT = att_e.tile([128, 1024], dt.bfloat16, tag="eT", name="eT")
                            nc.scalar.activation(eT[:], s_ps[:], AF.Exp, scale=float(SCALE))
                            for j in range(2):
                                kb = g * 2 + j
                                nc.tensor.matmul(
                                    o_accs[hh][:],
                                    v_full[:, kb * VW + 65 * (2 * t + hh): kb * VW + 65 * (2 * t + hh) + 65],
                                    eT[:, j * SH:(j + 1) * SH],
                                    start=(g == 0 and j == 0), stop=(g == 15 and j == 1))
                    for hh in range(2):
                        b = 64 * hh
                        rz = att_sm.tile([1, SH], dt.float32, tag="rz", name="rz")
                        nc.vector.reciprocal(rz[:], o_accs[hh][64:65, :])
                        rz_r = att_sm.tile([1, SH], dt.float32r, tag="rzr", name="rz_r")
                        nc.vector.tensor_copy(rz_r[:], rz[:])
                        z_ps = z_ps_pool.tile([64, SH], dt.float32, tag="zps", name="z_ps")
                        nc.tensor.matmul(z_ps[:], ones_r[:], rz_r[:], start=True, stop=True)
                        zbc = att_sm.tile([64, SH], dt.float32, tag="zbc", name="zbc")
                        nc.vector.tensor_copy(zbc[:], z_ps[:])
                        nc.vector.tensor_tensor(
                            out=oT[b:b + 64, t * SH:(t + 1) * SH],
                            in0=o_accs[hh][0:64, :], in1=zbc[:],
                            op=mybir.AluOpType.mult)

              # ---- o_proj + residual + rmsnorm2 (attention PSUM pools closed) ----
              with tc.tile_pool(name="workB", bufs=2) as workB, \
                   tc.tile_pool(name="psB", bufs=2, space="PSUM") as psB, \
                   tc.tile_pool(name="pstB", bufs=2, space="PSUM") as pstB:
                for sb in range(NSB):
                    for jc in range(2):
                        pp = psB.tile([128, 384], dt.float32, tag="projo", name="pp")
                        for dvt in range(NKT):
                            nc.tensor.matmul(pp[:], oT[:, dvt * SH + sb * 128: dvt * SH + (sb + 1) * 128],
                                             wo_all[:, dvt * D + jc * 384: dvt * D + (jc + 1) * 384],
                                             start=(dvt == 0), stop=(dvt == NKT - 1))
                        nc.vector.tensor_tensor(
                            out=x2_sb[:, sb * D + jc * 384: sb * D + (jc + 1) * 384],
                            in0=pp[:], in1=x_sb[:, sb * D + jc * 384: sb * D + (jc + 1) * 384],
                            op=mybir.AluOpType.add)
                    rmsnorm_to_xnT(nc, workB, pstB, ident,
                                   x2_sb[:, sb * D:(sb + 1) * D], xnT2, sb)

            # ============ phase C: MLP ============
            with tc.tile_pool(name="mlp_w", bufs=1) as mlp_w, \
                 tc.tile_pool(name="workC", bufs=3) as workC, \
                 tc.tile_pool(name="mlp_ps", bufs=2, space="PSUM") as mlp_ps:
                wd_all = mlp_w.tile([128, NIT * D], dt.bfloat16)
                for it in range(NIT):
                    nc.gpsimd.dma_start(wd_all[:, it * D:(it + 1) * D],
                                        WD.ap()[it * 128:(it + 1) * 128, :])
                sgT = mlp_w.tile([128, NIT * SH], dt.bfloat16)
                for it in range(NIT):
                    pp = mlp_ps.tile([128, SH], dt.float32, tag="up", name="pp")
                    for kt in range(NKT):
                        nc.tensor.matmul(pp[:], wup_all[:, kt * I + it * 128: kt * I + (it + 1) * 128],
                                         xnT2[:, kt * SH:(kt + 1) * SH],
                                         start=(kt == 0), stop=(kt == NKT - 1))
                    nc.scalar.activation(sgT[:, it * SH:(it + 1) * SH], pp[:], AF.Silu)
                for sb in range(NSB):
                    for jc in range(2):
                        pp = mlp_ps.tile([128, 384], dt.float32, tag="down", name="pp")
                        for it in range(NIT):
                            nc.tensor.matmul(pp[:], sgT[:, it * SH + sb * 128: it * SH + (sb + 1) * 128],
                                             wd_all[:, it * D + jc * 384: it * D + (jc + 1) * 384],
                                             start=(it == 0), stop=(it == NIT - 1))
                        o_sb = workC.tile([128, 384], dt.float32, tag="osb", name="o_sb")
                        nc.vector.tensor_tensor(
                            out=o_sb[:], in0=pp[:],
                            in1=x2_sb[:, sb * D + jc * 384: sb * D + (jc + 1) * 384],
                            op=mybir.AluOpType.add)
                        nc.sync.dma_start(OUT.ap()[sb * 128:(sb + 1) * 128, jc * 384:(jc + 1) * 384], o_sb[:])
    nc.compile()
    return nc


def _get_nc(reps=1):
    key = f"nc{reps}"
    if key not in _CACHED:
        _CACHED[key] = _build(reps)
    return _CACHED[key]


def _prep_in_maps(hidden_states, wq, wk, wv, wo, w_up, w_down, ln1_w, ln2_w):
    bf16 = ml_dtypes.bfloat16
    x = np.asarray(hidden_states, np.float32).reshape(S, D)
    wqT = np.ascontiguousarray((np.asarray(wq, np.float32) * np.asarray(ln1_w, np.float32)[None, :]).T).astype(bf16)
    wkT = np.ascontiguousarray((np.asarray(wk, np.float32) * np.asarray(ln1_w, np.float32)[None, :]).T).astype(bf16)
    wvT = np.ascontiguousarray((np.asarray(wv, np.float32) * np.asarray(ln1_w, np.float32)[None, :]).T).astype(bf16)
    woT = np.ascontiguousarray(np.asarray(wo, np.float32).T).astype(bf16)
    wupT = np.ascontiguousarray((np.asarray(w_up, np.float32) * np.asarray(ln2_w, np.float32)[None, :]).T).astype(bf16)
    wdownT = np.ascontiguousarray(np.asarray(w_down, np.float32).T).astype(bf16)
    return [{
        "x_shard": np.ascontiguousarray(x[c * SH:(c + 1) * SH]),
        "wqT": wqT, "wkT": wkT, "wvT": wvT, "woT": woT,
        "wupT": wupT, "wdownT": wdownT,
    } for c in range(N_CORES)]


def _get_runner():
    """Build the sharded jitted executable once; reuse across calls."""
    if "runner" in _CACHED:
        return _CACHED["runner"]
    import jax
    from jax.sharding import Mesh, PartitionSpec
    try:
        from jax.experimental.shard_map import shard_map
    except ImportError:
        shard_map = jax.shard_map
    from concourse import bass2jax
    bass2jax.install_neuronx_cc_hook()
    nc = _get_nc()
    import concourse.mybir as mybir_m
    partition_name = nc.partition_id_tensor.name if nc.partition_id_tensor else None
    in_names, out_names, out_avals, zero_outs = [], [], [], []
    for alloc in nc.m.functions[0].allocations:
        if not isinstance(alloc, mybir_m.MemoryLocationSet):
            continue
        name = alloc.memorylocations[0].name
        if alloc.kind == "ExternalInput":
            if name != partition_name:
                in_names.append(name)
        elif alloc.kind == "ExternalOutput":
            out_names.append(name)
            shape = tuple(alloc.tensor_shape)
            dtype = mybir_m.dt.np(alloc.dtype)
            out_avals.append(jax.core.ShapedArray(shape, dtype))
            zero_outs.append(np.zeros(shape, dtype))
    n_params = len(in_names)
    all_names = list(in_names) + list(out_names)
    if partition_name is not None:
        all_names.append(partition_name)

    def _body(*args):
        operands = list(args)
        if partition_name is not None:
            operands.append(bass2jax.partition_id_tensor())
        outs = bass2jax._bass_exec_p.bind(
            *operands, out_avals=tuple(out_avals), in_names=tuple(all_names),
            out_names=tuple(out_names), lowering_input_output_aliases=(),
            sim_require_finite=True, sim_require_nnan=True, nc=nc)
        return tuple(outs)

    devices = jax.devices()[:N_CORES]
    mesh = Mesh(np.asarray(devices), ("core",))
    in_specs = (PartitionSpec("core"),) * (n_params + len(out_names))
    out_specs = (PartitionSpec("core"),) * len(out_names)
    fn = jax.jit(shard_map(_body, mesh=mesh, in_specs=in_specs,
                           out_specs=out_specs, check_rep=False))

    def run(in_maps):
        concat_in = [np.concatenate([np.asarray(in_maps[c][n]) for c in range(N_CORES)], axis=0)
                     for n in in_names]
        concat_zero = [np.zeros((N_CORES * z.shape[0], *z.shape[1:]), z.dtype) for z in zero_outs]
        out_arrs = fn(*concat_in, *concat_zero)
        jax.block_until_ready(out_arrs)
        return {name: np.asarray(out_arrs[i]) for i, name in enumerate(out_names)}

    _CACHED["runner_parts"] = (fn, in_names, out_names, zero_outs, mesh)
    _CACHED["runner"] = run
    return run


def kernel(hidden_states, wq, wk, wv, wo, w_up, w_down, ln1_w, ln2_w):
    in_maps = _prep_in_maps(hidden_states, wq, wk, wv, wo, w_up, w_down, ln1_w, ln2_w)
    try:
        run = _get_runner()
        outs = run(in_maps)
        out = outs["out"].reshape(N_CORES, SH, D).reshape(S, D)
    except Exception:
        nc = _get_nc()
        res = run_bass_kernel_spmd(nc, in_maps, core_ids=list(range(N_CORES)))
        out = np.concatenate([res.results[c]["out"] for c in range(N_CORES)], axis=0)
    return out.reshape(1, S, D).astype(np.float32)



# revision 23
# speedup vs baseline: 1.1062x; 1.1062x over previous
"""Llama decoder layer (S=4096, D=768, NH=12, I=3072, fp32) on 8 TRN2 cores.

Strategy: sequence-sharded data parallel. Each core owns 512 rows:
  rmsnorm1 -> QKV (bf16 matmuls, ln folded into weights host-side; all
  weights uploaded as bf16 to halve HBM traffic)
  -> AllGather K^T and V (bf16) -> full-softmax attention in transposed
  layout (scoresT [k,q]; exp with no max-subtract — scores are O(1);
  per-head double-buffered 2-bank scores PSUM so exp overlaps the next
  scores matmul; AV accumulates in PSUM across all 16 k-groups in one
  start/stop chain; softmax denominator via a ones-column appended to V;
  Z broadcast via a tiny PE matmul) -> o_proj + residual -> rmsnorm2 ->
  MLP (silu) -> residual. The K/V AllGather is split in two: the K-gather
  is issued right after the K projection and overlaps the Q/V projection
  matmuls (scores need only K; AV needs V later). Host re-assembles the
  8 output shards. Marginal device time ~0.7ms/layer by reps=4 slope;
  split-AG measured -42us/layer vs single-AG in same-process A/B.
"""
import numpy as np
import ml_dtypes  # noqa: F401  (bf16 host checks)
import concourse.bass as bass
import concourse.tile as tile
from concourse import bacc, mybir
from concourse import masks
from concourse.bass_utils import run_bass_kernel_spmd

dt = mybir.dt
AF = mybir.ActivationFunctionType

N_CORES = 8
S, D, NH, HD, I = 4096, 768, 12, 64, 3072
SH = S // N_CORES          # 512 rows per core
NSB = SH // 128            # 4 s-blocks per core
NKT = D // 128             # 6 contraction tiles over D
NIT = I // 128             # 24 i-tiles
PAIRS = NH // 2            # 6 head pairs
VW = 65 * NH               # 780: V row width with ones col per head
KT_ELEMS = D * SH          # KT shard elems (bf16)
V_ELEMS = SH * VW
KV_ELEMS = KT_ELEMS + V_ELEMS
EPS = 1e-5
SCALE = 1.0 / np.sqrt(HD)

_CACHED = {}


def _build(reps=1):
    # reps>1 unrolls the whole layer body N times in one NEFF — used only
    # for slope-calibration timing (time(2x)-time(1x) cancels launch
    # overhead); kernel() always runs the reps=1 build.
    nc = bacc.Bacc("TRN2", target_bir_lowering=False, debug=False,
                   enable_asserts=False, num_devices=N_CORES)
    Xs = nc.dram_tensor("x_shard", [SH, D], dt.float32, kind="ExternalInput")
    WQ = nc.dram_tensor("wqT", [D, D], dt.bfloat16, kind="ExternalInput")
    WK = nc.dram_tensor("wkT", [D, D], dt.bfloat16, kind="ExternalInput")
    WV = nc.dram_tensor("wvT", [D, D], dt.bfloat16, kind="ExternalInput")
    WO = nc.dram_tensor("woT", [D, D], dt.bfloat16, kind="ExternalInput")
    WU = nc.dram_tensor("wupT", [D, I], dt.bfloat16, kind="ExternalInput")
    WD = nc.dram_tensor("wdownT", [I, D], dt.bfloat16, kind="ExternalInput")
    OUT = nc.dram_tensor("out", [SH, D], dt.float32, kind="ExternalOutput")

    def rmsnorm_to_xnT(nc, wk_pool, pst, ident, xsrc, xnT_dst, sb):
        sq = wk_pool.tile([128, D], dt.float32, tag="sq", name="sq")
        nc.vector.tensor_mul(sq[:], xsrc, xsrc)
        ssum = wk_pool.tile([128, 1], dt.float32, tag="ssum", name="ssum")
        nc.vector.reduce_sum(ssum[:], sq[:], axis=mybir.AxisListType.X)
        mvar = wk_pool.tile([128, 1], dt.float32, tag="mvar", name="mvar")
        nc.vector.tensor_scalar(out=mvar[:], in0=ssum[:], scalar1=1.0 / D,
                                scalar2=EPS, op0=mybir.AluOpType.mult,
                                op1=mybir.AluOpType.add)
        rvar = wk_pool.tile([128, 1], dt.float32, tag="rvar", name="rvar")
        nc.vector.reciprocal(rvar[:], mvar[:])
        rr = wk_pool.tile([128, 1], dt.float32, tag="rr", name="rr")
        nc.scalar.activation(rr[:], rvar[:], AF.Sqrt)
        xn = wk_pool.tile([128, D], dt.float32, tag="xn", name="xn")
        nc.vector.tensor_scalar_mul(xn[:], xsrc, rr[:])
        for kt in range(NKT):
            t_ps = pst.tile([128, 128], dt.float32, tag="tps", name="t_ps")
            nc.tensor.transpose(t_ps[:], xn[:, kt * 128:(kt + 1) * 128], ident[:])
            nc.vector.tensor_copy(xnT_dst[:, kt * SH + sb * 128: kt * SH + (sb + 1) * 128], t_ps[:])

    with tile.TileContext(nc) as tc:
      for rep in range(reps):
        with tc.tile_pool(name=f"const{rep}", bufs=1) as constp, \
             tc.tile_pool(name=f"mid{rep}", bufs=1) as mid, \
             tc.tile_pool(name=f"dram{rep}", bufs=1, space="DRAM") as dram:

            ident = constp.tile([128, 128], dt.float32)
            masks.make_identity(nc, ident[:])
            ones_f = constp.tile([1, 64], dt.float32)
            nc.gpsimd.memset(ones_f[:], 1.0)
            ones_r = constp.tile([1, 64], dt.float32r)
            nc.vector.tensor_copy(ones_r[:], ones_f[:])

            x_sb = mid.tile([128, NSB * D], dt.float32)
            x2_sb = mid.tile([128, NSB * D], dt.float32)
            qT = mid.tile([128, NKT * SH], dt.bfloat16)
            xnT2 = mid.tile([128, NKT * SH], dt.bfloat16)
            # MLP-up weights live in the outer pool and load during phase A:
            # the first up-proj matmul needs a slice of every kt-chunk, so a
            # phase-C load would expose the whole 4.7MB DMA.
            wup_all = mid.tile([128, NKT * I], dt.bfloat16)
            k_in = dram.tile([KT_ELEMS], dt.bfloat16)
            v_in = dram.tile([V_ELEMS], dt.bfloat16)
            k_all = dram.tile([N_CORES, KT_ELEMS], dt.bfloat16, addr_space="Shared")
            v_all = dram.tile([N_CORES, V_ELEMS], dt.bfloat16, addr_space="Shared")

            # ============ phase A: norm1, QKV, AG ============
            with tc.tile_pool(name="wqkv", bufs=1) as wqkv, \
                 tc.tile_pool(name="workA", bufs=2) as workA, \
                 tc.tile_pool(name="psA", bufs=2, space="PSUM") as psA, \
                 tc.tile_pool(name="pstA", bufs=2, space="PSUM") as pstA:
                wq_all = wqkv.tile([128, NKT * D], dt.bfloat16)
                wk_all = wqkv.tile([128, NKT * D], dt.bfloat16)
                wv_all = wqkv.tile([128, NKT * D], dt.bfloat16)
                for kt in range(NKT):
                    nc.gpsimd.dma_start(wq_all[:, kt * D:(kt + 1) * D], WQ.ap()[kt * 128:(kt + 1) * 128, :])
                    nc.gpsimd.dma_start(wk_all[:, kt * D:(kt + 1) * D], WK.ap()[kt * 128:(kt + 1) * 128, :])
                    nc.gpsimd.dma_start(wv_all[:, kt * D:(kt + 1) * D], WV.ap()[kt * 128:(kt + 1) * 128, :])
                for kt in range(NKT):
                    for ch in range(4):
                        nc.gpsimd.dma_start(
                            wup_all[:, kt * I + ch * 768: kt * I + (ch + 1) * 768],
                            WU.ap()[kt * 128:(kt + 1) * 128, ch * 768:(ch + 1) * 768])

                # V is the only per-s-block projection (x-stationary), so it
                # interleaves into the rmsnorm loop: the PE computes V(sb)
                # while the vector engine runs rmsnorm(sb+1), instead of
                # idling until all four blocks are normed.
                xnT = wqkv.tile([128, NKT * SH], dt.bfloat16)
                v_sh = wqkv.tile([128, NSB * VW], dt.bfloat16)
                nc.gpsimd.memset(v_sh[:], 1.0)
                for sb in range(NSB):
                    xs = x_sb[:, sb * D:(sb + 1) * D]
                    nc.sync.dma_start(xs, Xs.ap()[sb * 128:(sb + 1) * 128, :])
                    rmsnorm_to_xnT(nc, workA, pstA, ident, xs, xnT, sb)
                    for jc in range(2):
                        pp = psA.tile([128, 384], dt.float32, tag="projv", name="pp")
                        for kt in range(NKT):
                            nc.tensor.matmul(pp[:], xnT[:, kt * SH + sb * 128: kt * SH + (sb + 1) * 128],
                                             wv_all[:, kt * D + jc * 384: kt * D + (jc + 1) * 384],
                                             start=(kt == 0), stop=(kt == NKT - 1))
                        for h6 in range(6):
                            h = jc * 6 + h6
                            nc.vector.tensor_copy(
                                v_sh[:, sb * VW + 65 * h: sb * VW + 65 * h + 64],
                                pp[:, h6 * 64:(h6 + 1) * 64])

                # K projection first; its AllGather flies while Q and V
                # projections run on the PE (scores only need K, AV needs V
                # later — the big collective is no longer one barrier).
                kT = wqkv.tile([128, NKT * SH], dt.bfloat16)
                for ob in range(NKT):
                    pp = psA.tile([128, SH], dt.float32, tag="proj", name="pp")
                    for kt in range(NKT):
                        nc.tensor.matmul(pp[:], wk_all[:, kt * D + ob * 128: kt * D + (ob + 1) * 128],
                                         xnT[:, kt * SH:(kt + 1) * SH],
                                         start=(kt == 0), stop=(kt == NKT - 1))
                    nc.vector.tensor_copy(kT[:, ob * SH:(ob + 1) * SH], pp[:])
                for ob in range(NKT):
                    nc.sync.dma_start(
                        k_in[ob * 128 * SH:(ob + 1) * 128 * SH].rearrange("(p c) -> p c", p=128),
                        kT[:, ob * SH:(ob + 1) * SH])
                nc.gpsimd.collective_compute(
                    "AllGather", mybir.AluOpType.bypass,
                    replica_groups=[list(range(N_CORES))],
                    ins=[k_in.opt()], outs=[k_all.opt()])
                for ob in range(NKT):
                    pp = psA.tile([128, SH], dt.float32, tag="proj", name="pp")
                    for kt in range(NKT):
                        nc.tensor.matmul(pp[:], wq_all[:, kt * D + ob * 128: kt * D + (ob + 1) * 128],
                                         xnT[:, kt * SH:(kt + 1) * SH],
                                         start=(kt == 0), stop=(kt == NKT - 1))
                    nc.vector.tensor_copy(qT[:, ob * SH:(ob + 1) * SH], pp[:])
                for sb in range(NSB):
                    nc.sync.dma_start(
                        v_in[sb * 128 * VW: (sb + 1) * 128 * VW]
                        .rearrange("(p c) -> p c", p=128),
                        v_sh[:, sb * VW:(sb + 1) * VW])
            nc.gpsimd.collective_compute(
                "AllGather", mybir.AluOpType.bypass,
                replica_groups=[list(range(N_CORES))],
                ins=[v_in.opt()], outs=[v_all.opt()])

            # ============ phase B: attention + o_proj + norm2 ============
            with tc.tile_pool(name="attB", bufs=1) as attB:
              with tc.tile_pool(name="att_kt", bufs=2) as att_kt, \
                 tc.tile_pool(name="att_e", bufs=3) as att_e, \
                 tc.tile_pool(name="att_sm", bufs=2) as att_sm, \
                 tc.tile_pool(name="att_ps", bufs=2, space="PSUM") as att_ps, \
                 tc.tile_pool(name="o_ps_pool", bufs=1, space="PSUM") as o_ps_pool, \
                 tc.tile_pool(name="z_ps_pool", bufs=1, space="PSUM") as z_ps_pool:
                wo_all = attB.tile([128, NKT * D], dt.bfloat16)
                for kt in range(NKT):
                    nc.gpsimd.dma_start(wo_all[:, kt * D:(kt + 1) * D], WO.ap()[kt * 128:(kt + 1) * 128, :])
                v_full = attB.tile([128, 32 * VW], dt.bfloat16)
                for r in range(N_CORES):
                    for sb in range(NSB):
                        c = r * NSB + sb
                        nc.sync.dma_start(
                            v_full[:, c * VW:(c + 1) * VW],
                            v_all[r, sb * 128 * VW: (sb + 1) * 128 * VW]
                            .rearrange("(p c) -> p c", p=128))
                oT = attB.tile([128, PAIRS * SH], dt.bfloat16)
                for t in range(PAIRS):
                    kt_pair = att_kt.tile([128, S], dt.bfloat16, tag="ktp", name="kt_pair")
                    for r in range(N_CORES):
                        nc.sync.dma_start(
                            kt_pair[:, r * SH:(r + 1) * SH],
                            k_all[r, t * 128 * SH:(t + 1) * 128 * SH]
                            .rearrange("(p c) -> p c", p=128))
                    # AV accumulates directly in PSUM across all 16 k-groups
                    # (one start/stop chain per head) — no per-group SBUF
                    # round-trip / vector adds on the critical path.
                    o_acc0 = o_ps_pool.tile([65, SH], dt.float32, tag="oacc0", name="o_acc0", bufs=1)
                    o_acc1 = o_ps_pool.tile([65, SH], dt.float32, tag="oacc1", name="o_acc1", bufs=1)
                    o_accs = (o_acc0, o_acc1)
                    for g in range(16):
                        # per-head 2-bank scores psum, double-buffered: exp(g,hh)
                        # overlaps the PE's next scores matmul instead of
                        # serializing PE -> scalar -> PE each group.
                        for hh in range(2):
                            b = 64 * hh
                            s_ps = att_ps.tile([128, 1024], dt.float32, tag="sps", name="s_ps", bufs=2)
                            for j in range(2):
                                kb = g * 2 + j
                                nc.tensor.matmul(
                                    s_ps[:, j * SH:(j + 1) * SH],
                                    kt_pair[b:b + 64, kb * 128:(kb + 1) * 128],
                                    qT[b:b + 64, t * SH:(t + 1) * SH],
                                    start=True, stop=True)
                            eT = att_e.tile([128, 1024], dt.bfloat16, tag="eT", name="eT")
                            nc.scalar.activation(eT[:], s_ps[:], AF.Exp, scale=float(SCALE))
                            for j in range(2):
                                kb = g * 2 + j
                                nc.tensor.matmul(
                                    o_accs[hh][:],
                                    v_full[:, kb * VW + 65 * (2 * t + hh): kb * VW + 65 * (2 * t + hh) + 65],
                                    eT[:, j * SH:(j + 1) * SH],
                                    start=(g == 0 and j == 0), stop=(g == 15 and j == 1))
                    for hh in range(2):
                        b = 64 * hh
                        rz = att_sm.tile([1, SH], dt.float32, tag="rz", name="rz")
                        nc.vector.reciprocal(rz[:], o_accs[hh][64:65, :])
                        rz_r = att_sm.tile([1, SH], dt.float32r, tag="rzr", name="rz_r")
                        nc.vector.tensor_copy(rz_r[:], rz[:])
                        z_ps = z_ps_pool.tile([64, SH], dt.float32, tag="zps", name="z_ps")
                        nc.tensor.matmul(z_ps[:], ones_r[:], rz_r[:], start=True, stop=True)
                        zbc = att_sm.tile([64, SH], dt.float32, tag="zbc", name="zbc")
                        nc.vector.tensor_copy(zbc[:], z_ps[:])
                        nc.vector.tensor_tensor(
                            out=oT[b:b + 64, t * SH:(t + 1) * SH],
                            in0=o_accs[hh][0:64, :], in1=zbc[:],
                            op=mybir.AluOpType.mult)

              # ---- o_proj + residual + rmsnorm2 (attention PSUM pools closed) ----
              with tc.tile_pool(name="workB", bufs=2) as workB, \
                   tc.tile_pool(name="psB", bufs=2, space="PSUM") as psB, \
                   tc.tile_pool(name="pstB", bufs=2, space="PSUM") as pstB:
                for sb in range(NSB):
                    for jc in range(2):
                        pp = psB.tile([128, 384], dt.float32, tag="projo", name="pp")
                        for dvt in range(NKT):
                            nc.tensor.matmul(pp[:], oT[:, dvt * SH + sb * 128: dvt * SH + (sb + 1) * 128],
                                             wo_all[:, dvt * D + jc * 384: dvt * D + (jc + 1) * 384],
                                             start=(dvt == 0), stop=(dvt == NKT - 1))
                        nc.vector.tensor_tensor(
                            out=x2_sb[:, sb * D + jc * 384: sb * D + (jc + 1) * 384],
                            in0=pp[:], in1=x_sb[:, sb * D + jc * 384: sb * D + (jc + 1) * 384],
                            op=mybir.AluOpType.add)
                    rmsnorm_to_xnT(nc, workB, pstB, ident,
                                   x2_sb[:, sb * D:(sb + 1) * D], xnT2, sb)

            # ============ phase C: MLP ============
            with tc.tile_pool(name="mlp_w", bufs=1) as mlp_w, \
                 tc.tile_pool(name="workC", bufs=3) as workC, \
                 tc.tile_pool(name="mlp_ps", bufs=2, space="PSUM") as mlp_ps:
                wd_all = mlp_w.tile([128, NIT * D], dt.bfloat16)
                for it in range(NIT):
                    nc.gpsimd.dma_start(wd_all[:, it * D:(it + 1) * D],
                                        WD.ap()[it * 128:(it + 1) * 128, :])
                sgT = mlp_w.tile([128, NIT * SH], dt.bfloat16)
                for it in range(NIT):
                    pp = mlp_ps.tile([128, SH], dt.float32, tag="up", name="pp")
                    for kt in range(NKT):
                        nc.tensor.matmul(pp[:], wup_all[:, kt * I + it * 128: kt * I + (it + 1) * 128],
                                         xnT2[:, kt * SH:(kt + 1) * SH],
                                         start=(kt == 0), stop=(kt == NKT - 1))
                    nc.scalar.activation(sgT[:, it * SH:(it + 1) * SH], pp[:], AF.Silu)
                for sb in range(NSB):
                    for jc in range(2):
                        pp = mlp_ps.tile([128, 384], dt.float32, tag="down", name="pp")
                        for it in range(NIT):
                            nc.tensor.matmul(pp[:], sgT[:, it * SH + sb * 128: it * SH + (sb + 1) * 128],
                                             wd_all[:, it * D + jc * 384: it * D + (jc + 1) * 384],
                                             start=(it == 0), stop=(it == NIT - 1))
                        o_sb = workC.tile([128, 384], dt.float32, tag="osb", name="o_sb")
                        nc.vector.tensor_tensor(
                            out=o_sb[:], in0=pp[:],
                            in1=x2_sb[:, sb * D + jc * 384: sb * D + (jc + 1) * 384],
                            op=mybir.AluOpType.add)
                        nc.sync.dma_start(OUT.ap()[sb * 128:(sb + 1) * 128, jc * 384:(jc + 1) * 384], o_sb[:])
    nc.compile()
    return nc


def _get_nc(reps=1):
    key = f"nc{reps}"
    if key not in _CACHED:
        _CACHED[key] = _build(reps)
    return _CACHED[key]


def _prep_in_maps(hidden_states, wq, wk, wv, wo, w_up, w_down, ln1_w, ln2_w):
    bf16 = ml_dtypes.bfloat16
    x = np.asarray(hidden_states, np.float32).reshape(S, D)
    wqT = np.ascontiguousarray((np.asarray(wq, np.float32) * np.asarray(ln1_w, np.float32)[None, :]).T).astype(bf16)
    wkT = np.ascontiguousarray((np.asarray(wk, np.float32) * np.asarray(ln1_w, np.float32)[None, :]).T).astype(bf16)
    wvT = np.ascontiguousarray((np.asarray(wv, np.float32) * np.asarray(ln1_w, np.float32)[None, :]).T).astype(bf16)
    woT = np.ascontiguousarray(np.asarray(wo, np.float32).T).astype(bf16)
    wupT = np.ascontiguousarray((np.asarray(w_up, np.float32) * np.asarray(ln2_w, np.float32)[None, :]).T).astype(bf16)
    wdownT = np.ascontiguousarray(np.asarray(w_down, np.float32).T).astype(bf16)
    return [{
        "x_shard": np.ascontiguousarray(x[c * SH:(c + 1) * SH]),
        "wqT": wqT, "wkT": wkT, "wvT": wvT, "woT": woT,
        "wupT": wupT, "wdownT": wdownT,
    } for c in range(N_CORES)]


def _get_runner():
    """Build the sharded jitted executable once; reuse across calls."""
    if "runner" in _CACHED:
        return _CACHED["runner"]
    import jax
    from jax.sharding import Mesh, PartitionSpec
    try:
        from jax.experimental.shard_map import shard_map
    except ImportError:
        shard_map = jax.shard_map
    from concourse import bass2jax
    bass2jax.install_neuronx_cc_hook()
    nc = _get_nc()
    import concourse.mybir as mybir_m
    partition_name = nc.partition_id_tensor.name if nc.partition_id_tensor else None
    in_names, out_names, out_avals, zero_outs = [], [], [], []
    for alloc in nc.m.functions[0].allocations:
        if not isinstance(alloc, mybir_m.MemoryLocationSet):
            continue
        name = alloc.memorylocations[0].name
        if alloc.kind == "ExternalInput":
            if name != partition_name:
                in_names.append(name)
        elif alloc.kind == "ExternalOutput":
            out_names.append(name)
            shape = tuple(alloc.tensor_shape)
            dtype = mybir_m.dt.np(alloc.dtype)
            out_avals.append(jax.core.ShapedArray(shape, dtype))
            zero_outs.append(np.zeros(shape, dtype))
    n_params = len(in_names)
    all_names = list(in_names) + list(out_names)
    if partition_name is not None:
        all_names.append(partition_name)

    def _body(*args):
        operands = list(args)
        if partition_name is not None:
            operands.append(bass2jax.partition_id_tensor())
        outs = bass2jax._bass_exec_p.bind(
            *operands, out_avals=tuple(out_avals), in_names=tuple(all_names),
            out_names=tuple(out_names), lowering_input_output_aliases=(),
            sim_require_finite=True, sim_require_nnan=True, nc=nc)
        return tuple(outs)

    devices = jax.devices()[:N_CORES]
    mesh = Mesh(np.asarray(devices), ("core",))
    in_specs = (PartitionSpec("core"),) * (n_params + len(out_names))
    out_specs = (PartitionSpec("core"),) * len(out_names)
    fn = jax.jit(shard_map(_body, mesh=mesh, in_specs=in_specs,
                           out_specs=out_specs, check_rep=False))

    def run(in_maps):
        concat_in = [np.concatenate([np.asarray(in_maps[c][n]) for c in range(N_CORES)], axis=0)
                     for n in in_names]
        concat_zero = [np.zeros((N_CORES * z.shape[0], *z.shape[1:]), z.dtype) for z in zero_outs]
        out_arrs = fn(*concat_in, *concat_zero)
        jax.block_until_ready(out_arrs)
        return {name: np.asarray(out_arrs[i]) for i, name in enumerate(out_names)}

    _CACHED["runner_parts"] = (fn, in_names, out_names, zero_outs, mesh)
    _CACHED["runner"] = run
    return run


def kernel(hidden_states, wq, wk, wv, wo, w_up, w_down, ln1_w, ln2_w):
    in_maps = _prep_in_maps(hidden_states, wq, wk, wv, wo, w_up, w_down, ln1_w, ln2_w)
    try:
        run = _get_runner()
        outs = run(in_maps)
        out = outs["out"].reshape(N_CORES, SH, D).reshape(S, D)
    except Exception:
        nc = _get_nc()
        res = run_bass_kernel_spmd(nc, in_maps, core_ids=list(range(N_CORES)))
        out = np.concatenate([res.results[c]["out"] for c in range(N_CORES)], axis=0)
    return out.reshape(1, S, D).astype(np.float32)



# revision 25
# speedup vs baseline: 1.1309x; 1.0223x over previous
"""Llama decoder layer (S=4096, D=768, NH=12, I=3072, fp32) on 8 TRN2 cores.

Strategy: sequence-sharded data parallel. Each core owns 512 rows:
  rmsnorm1 -> QKV (bf16 matmuls, ln folded into weights host-side; all
  weights uploaded as bf16 to halve HBM traffic)
  -> AllGather K^T and V (bf16) -> full-softmax attention in transposed
  layout (scoresT [k,q]; exp with no max-subtract — scores are O(1);
  per-head double-buffered 2-bank scores PSUM so exp overlaps the next
  scores matmul; AV accumulates in PSUM across all 16 k-groups in one
  start/stop chain; softmax denominator via a ones-column appended to V;
  Z broadcast via a tiny PE matmul) -> o_proj + residual -> rmsnorm2 ->
  MLP (silu) -> residual. The K/V AllGather is split in two: the K-gather
  is issued right after the K projection and overlaps the Q/V projection
  matmuls (scores need only K; AV needs V later). Host re-assembles the
  8 output shards. Marginal device time ~0.7ms/layer by reps=4 slope;
  split-AG measured -42us/layer vs single-AG in same-process A/B.
"""
import numpy as np
import ml_dtypes  # noqa: F401  (bf16 host checks)
import concourse.bass as bass
import concourse.tile as tile
from concourse import bacc, mybir
from concourse import masks
from concourse.bass_utils import run_bass_kernel_spmd

dt = mybir.dt
AF = mybir.ActivationFunctionType

N_CORES = 8
S, D, NH, HD, I = 4096, 768, 12, 64, 3072
SH = S // N_CORES          # 512 rows per core
NSB = SH // 128            # 4 s-blocks per core
NKT = D // 128             # 6 contraction tiles over D
NIT = I // 128             # 24 i-tiles
PAIRS = NH // 2            # 6 head pairs
VW = 65 * NH               # 780: V row width with ones col per head
KT_ELEMS = D * SH          # KT shard elems (bf16)
V_ELEMS = SH * VW
KV_ELEMS = KT_ELEMS + V_ELEMS
EPS = 1e-5
SCALE = 1.0 / np.sqrt(HD)

_CACHED = {}


def _build(reps=1):
    # reps>1 unrolls the whole layer body N times in one NEFF — used only
    # for slope-calibration timing (time(2x)-time(1x) cancels launch
    # overhead); kernel() always runs the reps=1 build.
    nc = bacc.Bacc("TRN2", target_bir_lowering=False, debug=False,
                   enable_asserts=False, num_devices=N_CORES)
    Xs = nc.dram_tensor("x_shard", [SH, D], dt.float32, kind="ExternalInput")
    WQ = nc.dram_tensor("wqT", [D, D], dt.bfloat16, kind="ExternalInput")
    WK = nc.dram_tensor("wkT", [D, D], dt.bfloat16, kind="ExternalInput")
    WV = nc.dram_tensor("wvT", [D, D], dt.bfloat16, kind="ExternalInput")
    WO = nc.dram_tensor("woT", [D, D], dt.bfloat16, kind="ExternalInput")
    WU = nc.dram_tensor("wupT", [D, I], dt.bfloat16, kind="ExternalInput")
    WD = nc.dram_tensor("wdownT", [I, D], dt.bfloat16, kind="ExternalInput")
    OUT = nc.dram_tensor("out", [SH, D], dt.float32, kind="ExternalOutput")

    def rmsnorm_to_xnT(nc, wk_pool, pst, ident, xsrc, xnT_dst, sb):
        sq = wk_pool.tile([128, D], dt.float32, tag="sq", name="sq")
        nc.vector.tensor_mul(sq[:], xsrc, xsrc)
        ssum = wk_pool.tile([128, 1], dt.float32, tag="ssum", name="ssum")
        nc.vector.reduce_sum(ssum[:], sq[:], axis=mybir.AxisListType.X)
        mvar = wk_pool.tile([128, 1], dt.float32, tag="mvar", name="mvar")
        nc.vector.tensor_scalar(out=mvar[:], in0=ssum[:], scalar1=1.0 / D,
                                scalar2=EPS, op0=mybir.AluOpType.mult,
                                op1=mybir.AluOpType.add)
        rvar = wk_pool.tile([128, 1], dt.float32, tag="rvar", name="rvar")
        nc.vector.reciprocal(rvar[:], mvar[:])
        rr = wk_pool.tile([128, 1], dt.float32, tag="rr", name="rr")
        nc.scalar.activation(rr[:], rvar[:], AF.Sqrt)
        xn = wk_pool.tile([128, D], dt.float32, tag="xn", name="xn")
        nc.vector.tensor_scalar_mul(xn[:], xsrc, rr[:])
        for kt in range(NKT):
            t_ps = pst.tile([128, 128], dt.float32, tag="tps", name="t_ps")
            nc.tensor.transpose(t_ps[:], xn[:, kt * 128:(kt + 1) * 128], ident[:])
            nc.vector.tensor_copy(xnT_dst[:, kt * SH + sb * 128: kt * SH + (sb + 1) * 128], t_ps[:])

    with tile.TileContext(nc) as tc:
      for rep in range(reps):
        with tc.tile_pool(name=f"const{rep}", bufs=1) as constp, \
             tc.tile_pool(name=f"mid{rep}", bufs=1) as mid, \
             tc.tile_pool(name=f"dram{rep}", bufs=1, space="DRAM") as dram:

            ident = constp.tile([128, 128], dt.float32)
            masks.make_identity(nc, ident[:])
            ones_f = constp.tile([1, 64], dt.float32)
            nc.gpsimd.memset(ones_f[:], 1.0)
            ones_r = constp.tile([1, 64], dt.float32r)
            nc.vector.tensor_copy(ones_r[:], ones_f[:])

            x_sb = mid.tile([128, NSB * D], dt.float32)
            x2_sb = mid.tile([128, NSB * D], dt.float32)
            qT = mid.tile([128, NKT * SH], dt.bfloat16)
            xnT2 = mid.tile([128, NKT * SH], dt.bfloat16)
            # MLP-up weights live in the outer pool and load during phase A:
            # the first up-proj matmul needs a slice of every kt-chunk, so a
            # phase-C load would expose the whole 4.7MB DMA.
            wup_all = mid.tile([128, NKT * I], dt.bfloat16)
            k_in = dram.tile([KT_ELEMS], dt.bfloat16)
            v_in = dram.tile([V_ELEMS], dt.bfloat16)
            k_all = dram.tile([N_CORES, KT_ELEMS], dt.bfloat16, addr_space="Shared")
            v_all = dram.tile([N_CORES, V_ELEMS], dt.bfloat16, addr_space="Shared")

            # ============ phase A: norm1, QKV, AG ============
            with tc.tile_pool(name="wqkv", bufs=1) as wqkv, \
                 tc.tile_pool(name="workA", bufs=2) as workA, \
                 tc.tile_pool(name="psA", bufs=2, space="PSUM") as psA, \
                 tc.tile_pool(name="pstA", bufs=2, space="PSUM") as pstA:
                wq_all = wqkv.tile([128, NKT * D], dt.bfloat16)
                wk_all = wqkv.tile([128, NKT * D], dt.bfloat16)
                wv_all = wqkv.tile([128, NKT * D], dt.bfloat16)
                for kt in range(NKT):
                    nc.gpsimd.dma_start(wq_all[:, kt * D:(kt + 1) * D], WQ.ap()[kt * 128:(kt + 1) * 128, :])
                    nc.gpsimd.dma_start(wk_all[:, kt * D:(kt + 1) * D], WK.ap()[kt * 128:(kt + 1) * 128, :])
                    nc.gpsimd.dma_start(wv_all[:, kt * D:(kt + 1) * D], WV.ap()[kt * 128:(kt + 1) * 128, :])
                for kt in range(NKT):
                    for ch in range(4):
                        nc.gpsimd.dma_start(
                            wup_all[:, kt * I + ch * 768: kt * I + (ch + 1) * 768],
                            WU.ap()[kt * 128:(kt + 1) * 128, ch * 768:(ch + 1) * 768])

                # V is the only per-s-block projection (x-stationary), so it
                # interleaves into the rmsnorm loop: the PE computes V(sb)
                # while the vector engine runs rmsnorm(sb+1), instead of
                # idling until all four blocks are normed.
                xnT = wqkv.tile([128, NKT * SH], dt.bfloat16)
                v_sh = wqkv.tile([128, NSB * VW], dt.bfloat16)
                nc.gpsimd.memset(v_sh[:], 1.0)
                for sb in range(NSB):
                    xs = x_sb[:, sb * D:(sb + 1) * D]
                    nc.sync.dma_start(xs, Xs.ap()[sb * 128:(sb + 1) * 128, :])
                    rmsnorm_to_xnT(nc, workA, pstA, ident, xs, xnT, sb)
                    for jc in range(2):
                        pp = psA.tile([128, 384], dt.float32, tag="projv", name="pp")
                        for kt in range(NKT):
                            nc.tensor.matmul(pp[:], xnT[:, kt * SH + sb * 128: kt * SH + (sb + 1) * 128],
                                             wv_all[:, kt * D + jc * 384: kt * D + (jc + 1) * 384],
                                             start=(kt == 0), stop=(kt == NKT - 1))
                        for h6 in range(6):
                            h = jc * 6 + h6
                            nc.vector.tensor_copy(
                                v_sh[:, sb * VW + 65 * h: sb * VW + 65 * h + 64],
                                pp[:, h6 * 64:(h6 + 1) * 64])

                # K projection first; its AllGather flies while Q and V
                # projections run on the PE (scores only need K, AV needs V
                # later — the big collective is no longer one barrier).
                kT = wqkv.tile([128, NKT * SH], dt.bfloat16)
                for ob in range(NKT):
                    pp = psA.tile([128, SH], dt.float32, tag="proj", name="pp")
                    for kt in range(NKT):
                        nc.tensor.matmul(pp[:], wk_all[:, kt * D + ob * 128: kt * D + (ob + 1) * 128],
                                         xnT[:, kt * SH:(kt + 1) * SH],
                                         start=(kt == 0), stop=(kt == NKT - 1))
                    nc.vector.tensor_copy(kT[:, ob * SH:(ob + 1) * SH], pp[:])
                for ob in range(NKT):
                    nc.sync.dma_start(
                        k_in[ob * 128 * SH:(ob + 1) * 128 * SH].rearrange("(p c) -> p c", p=128),
                        kT[:, ob * SH:(ob + 1) * SH])
                nc.gpsimd.collective_compute(
                    "AllGather", mybir.AluOpType.bypass,
                    replica_groups=[list(range(N_CORES))],
                    ins=[k_in.opt()], outs=[k_all.opt()])
                for ob in range(NKT):
                    pp = psA.tile([128, SH], dt.float32, tag="proj", name="pp")
                    for kt in range(NKT):
                        nc.tensor.matmul(pp[:], wq_all[:, kt * D + ob * 128: kt * D + (ob + 1) * 128],
                                         xnT[:, kt * SH:(kt + 1) * SH],
                                         start=(kt == 0), stop=(kt == NKT - 1))
                    nc.vector.tensor_copy(qT[:, ob * SH:(ob + 1) * SH], pp[:])
                for sb in range(NSB):
                    nc.sync.dma_start(
                        v_in[sb * 128 * VW: (sb + 1) * 128 * VW]
                        .rearrange("(p c) -> p c", p=128),
                        v_sh[:, sb * VW:(sb + 1) * VW])
            nc.gpsimd.collective_compute(
                "AllGather", mybir.AluOpType.bypass,
                replica_groups=[list(range(N_CORES))],
                ins=[v_in.opt()], outs=[v_all.opt()])

            # ============ phase B: attention + o_proj + norm2 ============
            with tc.tile_pool(name="attB", bufs=1) as attB:
              with tc.tile_pool(name="att_kt", bufs=2) as att_kt, \
                 tc.tile_pool(name="att_e", bufs=3) as att_e, \
                 tc.tile_pool(name="att_sm", bufs=2) as att_sm, \
                 tc.tile_pool(name="att_ps", bufs=2, space="PSUM") as att_ps, \
                 tc.tile_pool(name="o_ps_pool", bufs=1, space="PSUM") as o_ps_pool, \
                 tc.tile_pool(name="z_ps_pool", bufs=1, space="PSUM") as z_ps_pool:
                wo_all = attB.tile([128, NKT * D], dt.bfloat16)
                for kt in range(NKT):
                    nc.gpsimd.dma_start(wo_all[:, kt * D:(kt + 1) * D], WO.ap()[kt * 128:(kt + 1) * 128, :])
                v_full = attB.tile([128, 32 * VW], dt.bfloat16)
                for r in range(N_CORES):
                    for sb in range(NSB):
                        c = r * NSB + sb
                        nc.sync.dma_start(
                            v_full[:, c * VW:(c + 1) * VW],
                            v_all[r, sb * 128 * VW: (sb + 1) * 128 * VW]
                            .rearrange("(p c) -> p c", p=128))
                oT = attB.tile([128, PAIRS * SH], dt.bfloat16)
                for t in range(PAIRS):
                    kt_pair = att_kt.tile([128, S], dt.bfloat16, tag="ktp", name="kt_pair")
                    for r in range(N_CORES):
                        nc.sync.dma_start(
                            kt_pair[:, r * SH:(r + 1) * SH],
                            k_all[r, t * 128 * SH:(t + 1) * 128 * SH]
                            .rearrange("(p c) -> p c", p=128))
                    # AV accumulates directly in PSUM across all 16 k-groups
                    # (one start/stop chain per head) — no per-group SBUF
                    # round-trip / vector adds on the critical path.
                    o_acc0 = o_ps_pool.tile([65, SH], dt.float32, tag="oacc0", name="o_acc0", bufs=1)
                    o_acc1 = o_ps_pool.tile([65, SH], dt.float32, tag="oacc1", name="o_acc1", bufs=1)
                    o_accs = (o_acc0, o_acc1)
                    for g in range(16):
                        # per-head 2-bank scores psum, double-buffered: exp(g,hh)
                        # overlaps the PE's next scores matmul instead of
                        # serializing PE -> scalar -> PE each group.
                        for hh in range(2):
                            b = 64 * hh
                            s_ps = att_ps.tile([128, 1024], dt.float32, tag="sps", name="s_ps", bufs=2)
                            for j in range(2):
                                kb = g * 2 + j
                                nc.tensor.matmul(
                                    s_ps[:, j * SH:(j + 1) * SH],
                                    kt_pair[b:b + 64, kb * 128:(kb + 1) * 128],
                                    qT[b:b + 64, t * SH:(t + 1) * SH],
                                    start=True, stop=True)
                            eT = att_e.tile([128, 1024], dt.bfloat16, tag="eT", name="eT")
                            nc.scalar.activation(eT[:], s_ps[:], AF.Exp, scale=float(SCALE))
                            for j in range(2):
                                kb = g * 2 + j
                                nc.tensor.matmul(
                                    o_accs[hh][:],
                                    v_full[:, kb * VW + 65 * (2 * t + hh): kb * VW + 65 * (2 * t + hh) + 65],
                                    eT[:, j * SH:(j + 1) * SH],
                                    start=(g == 0 and j == 0), stop=(g == 15 and j == 1))
                    for hh in range(2):
                        b = 64 * hh
                        rz = att_sm.tile([1, SH], dt.float32, tag="rz", name="rz")
                        nc.vector.reciprocal(rz[:], o_accs[hh][64:65, :])
                        rz_r = att_sm.tile([1, SH], dt.float32r, tag="rzr", name="rz_r")
                        nc.vector.tensor_copy(rz_r[:], rz[:])
                        z_ps = z_ps_pool.tile([64, SH], dt.float32, tag="zps", name="z_ps")
                        nc.tensor.matmul(z_ps[:], ones_r[:], rz_r[:], start=True, stop=True)
                        zbc = att_sm.tile([64, SH], dt.float32, tag="zbc", name="zbc")
                        nc.vector.tensor_copy(zbc[:], z_ps[:])
                        nc.vector.tensor_tensor(
                            out=oT[b:b + 64, t * SH:(t + 1) * SH],
                            in0=o_accs[hh][0:64, :], in1=zbc[:],
                            op=mybir.AluOpType.mult)

              # ---- o_proj + residual + rmsnorm2 (attention PSUM pools closed) ----
              with tc.tile_pool(name="workB", bufs=2) as workB, \
                   tc.tile_pool(name="psB", bufs=2, space="PSUM") as psB, \
                   tc.tile_pool(name="pstB", bufs=2, space="PSUM") as pstB:
                for sb in range(NSB):
                    for jc in range(2):
                        pp = psB.tile([128, 384], dt.float32, tag="projo", name="pp")
                        for dvt in range(NKT):
                            nc.tensor.matmul(pp[:], oT[:, dvt * SH + sb * 128: dvt * SH + (sb + 1) * 128],
                                             wo_all[:, dvt * D + jc * 384: dvt * D + (jc + 1) * 384],
                                             start=(dvt == 0), stop=(dvt == NKT - 1))
                        nc.vector.tensor_tensor(
                            out=x2_sb[:, sb * D + jc * 384: sb * D + (jc + 1) * 384],
                            in0=pp[:], in1=x_sb[:, sb * D + jc * 384: sb * D + (jc + 1) * 384],
                            op=mybir.AluOpType.add)
                    rmsnorm_to_xnT(nc, workB, pstB, ident,
                                   x2_sb[:, sb * D:(sb + 1) * D], xnT2, sb)

            # ============ phase C: MLP ============
            with tc.tile_pool(name="mlp_w", bufs=1) as mlp_w, \
                 tc.tile_pool(name="workC", bufs=3) as workC, \
                 tc.tile_pool(name="mlp_ps", bufs=2, space="PSUM") as mlp_ps:
                wd_all = mlp_w.tile([128, NIT * D], dt.bfloat16)
                for it in range(NIT):
                    nc.gpsimd.dma_start(wd_all[:, it * D:(it + 1) * D],
                                        WD.ap()[it * 128:(it + 1) * 128, :])
                sgT = mlp_w.tile([128, NIT * SH], dt.bfloat16)
                for it in range(NIT):
                    pp = mlp_ps.tile([128, SH], dt.float32, tag="up", name="pp")
                    for kt in range(NKT):
                        nc.tensor.matmul(pp[:], wup_all[:, kt * I + it * 128: kt * I + (it + 1) * 128],
                                         xnT2[:, kt * SH:(kt + 1) * SH],
                                         start=(kt == 0), stop=(kt == NKT - 1))
                    nc.scalar.activation(sgT[:, it * SH:(it + 1) * SH], pp[:], AF.Silu)
                for sb in range(NSB):
                    for jc in range(2):
                        pp = mlp_ps.tile([128, 384], dt.float32, tag="down", name="pp")
                        for it in range(NIT):
                            nc.tensor.matmul(pp[:], sgT[:, it * SH + sb * 128: it * SH + (sb + 1) * 128],
                                             wd_all[:, it * D + jc * 384: it * D + (jc + 1) * 384],
                                             start=(it == 0), stop=(it == NIT - 1))
                        o_sb = workC.tile([128, 384], dt.float32, tag="osb", name="o_sb")
                        nc.vector.tensor_tensor(
                            out=o_sb[:], in0=pp[:],
                            in1=x2_sb[:, sb * D + jc * 384: sb * D + (jc + 1) * 384],
                            op=mybir.AluOpType.add)
                        nc.sync.dma_start(OUT.ap()[sb * 128:(sb + 1) * 128, jc * 384:(jc + 1) * 384], o_sb[:])
    nc.compile()
    return nc


def _get_nc(reps=1):
    key = f"nc{reps}"
    if key not in _CACHED:
        _CACHED[key] = _build(reps)
    return _CACHED[key]


def _prep_in_maps(hidden_states, wq, wk, wv, wo, w_up, w_down, ln1_w, ln2_w):
    bf16 = ml_dtypes.bfloat16
    x = np.asarray(hidden_states, np.float32).reshape(S, D)
    wqT = np.ascontiguousarray((np.asarray(wq, np.float32) * np.asarray(ln1_w, np.float32)[None, :]).T).astype(bf16)
    wkT = np.ascontiguousarray((np.asarray(wk, np.float32) * np.asarray(ln1_w, np.float32)[None, :]).T).astype(bf16)
    wvT = np.ascontiguousarray((np.asarray(wv, np.float32) * np.asarray(ln1_w, np.float32)[None, :]).T).astype(bf16)
    woT = np.ascontiguousarray(np.asarray(wo, np.float32).T).astype(bf16)
    wupT = np.ascontiguousarray((np.asarray(w_up, np.float32) * np.asarray(ln2_w, np.float32)[None, :]).T).astype(bf16)
    wdownT = np.ascontiguousarray(np.asarray(w_down, np.float32).T).astype(bf16)
    return [{
        "x_shard": np.ascontiguousarray(x[c * SH:(c + 1) * SH]),
        "wqT": wqT, "wkT": wkT, "wvT": wvT, "woT": woT,
        "wupT": wupT, "wdownT": wdownT,
    } for c in range(N_CORES)]


def _get_runner():
    """Build the sharded jitted executable once; reuse across calls."""
    if "runner" in _CACHED:
        return _CACHED["runner"]
    import jax
    from jax.sharding import Mesh, PartitionSpec
    try:
        from jax.experimental.shard_map import shard_map
    except ImportError:
        shard_map = jax.shard_map
    from concourse import bass2jax
    bass2jax.install_neuronx_cc_hook()
    nc = _get_nc()
    import concourse.mybir as mybir_m
    partition_name = nc.partition_id_tensor.name if nc.partition_id_tensor else None
    in_names, out_names, out_avals, zero_outs = [], [], [], []
    for alloc in nc.m.functions[0].allocations:
        if not isinstance(alloc, mybir_m.MemoryLocationSet):
            continue
        name = alloc.memorylocations[0].name
        if alloc.kind == "ExternalInput":
            if name != partition_name:
                in_names.append(name)
        elif alloc.kind == "ExternalOutput":
            out_names.append(name)
            shape = tuple(alloc.tensor_shape)
            dtype = mybir_m.dt.np(alloc.dtype)
            out_avals.append(jax.core.ShapedArray(shape, dtype))
            zero_outs.append(np.zeros(shape, dtype))
    n_params = len(in_names)
    all_names = list(in_names) + list(out_names)
    if partition_name is not None:
        all_names.append(partition_name)

    def _body(*args):
        operands = list(args)
        if partition_name is not None:
            operands.append(bass2jax.partition_id_tensor())
        outs = bass2jax._bass_exec_p.bind(
            *operands, out_avals=tuple(out_avals), in_names=tuple(all_names),
            out_names=tuple(out_names), lowering_input_output_aliases=(),
            sim_require_finite=True, sim_require_nnan=True, nc=nc)
        return tuple(outs)

    devices = jax.devices()[:N_CORES]
    mesh = Mesh(np.asarray(devices), ("core",))
    in_specs = (PartitionSpec("core"),) * (n_params + len(out_names))
    out_specs = (PartitionSpec("core"),) * len(out_names)
    fn = jax.jit(shard_map(_body, mesh=mesh, in_specs=in_specs,
                           out_specs=out_specs, check_rep=False))

    def run(in_maps):
        concat_in = [np.concatenate([np.asarray(in_maps[c][n]) for c in range(N_CORES)], axis=0)
                     for n in in_names]
        concat_zero = [np.zeros((N_CORES * z.shape[0], *z.shape[1:]), z.dtype) for z in zero_outs]
        out_arrs = fn(*concat_in, *concat_zero)
        jax.block_until_ready(out_arrs)
        return {name: np.asarray(out_arrs[i]) for i, name in enumerate(out_names)}

    _CACHED["runner_parts"] = (fn, in_names, out_names, zero_outs, mesh)
    _CACHED["runner"] = run
    return run


def kernel(hidden_states, wq, wk, wv, wo, w_up, w_down, ln1_w, ln2_w):
    in_maps = _prep_in_maps(hidden_states, wq, wk, wv, wo, w_up, w_down, ln1_w, ln2_w)
    try:
        run = _get_runner()
        outs = run(in_maps)
        out = outs["out"].reshape(N_CORES, SH, D).reshape(S, D)
    except Exception:
        nc = _get_nc()
        res = run_bass_kernel_spmd(nc, in_maps, core_ids=list(range(N_CORES)))
        out = np.concatenate([res.results[c]["out"] for c in range(N_CORES)], axis=0)
    return out.reshape(1, S, D).astype(np.float32)

